# revision 29
# baseline (speedup 1.0000x reference)
"""BertSelfAttention on 8 Trainium2 NeuronCores (Bass/Tile, SPMD, no collectives).

Problem: hidden_states [2, 2048, 1024], 16 heads x 64 dims, causal_bias added
along the key axis before softmax.

Sharding: core c handles batch b = c//4 and head-group g = c%4 (4 heads, i.e.
256 of the 1024 projection dims).  Pure SPMD - every core runs the same
program on its own slice; the host does the (free) slicing / transposes and
the final gather.

Per-core device algorithm (all matmuls in fp32r = full-rate fp32):
  QT[m, s] = Wq_g @ hsT + bq   (m = 256 local head dims, s = 2048 positions)
  KT[m, s] = Wk_g @ hsT + bk
  V [s, m] = (hs @ Wv_g.T) * expb[s]   (expb = exp(causal_bias), no bv)
  per head h (2 row-packed pairs):
    sT[k, sq]  = KT_h.T @ QT_h          (scores transposed, k = key pos)
    P [k, sq]  = exp(sT * 0.125)        (bias folded in via expb; no max
                                         subtraction needed: |s/8| < ~3)
    ctxu[65, sq] += [V'_h | expb].T @ P (rows 0..63 = unnormalized ctx^T,
                                         row 64 = softmax denominator)
  DMA ctxu to DRAM.
Host: ctx = (ctxu[:64] / ctxu[64]).T + bv  and scatter into [B, S, H].

The exp(bias) folding works because softmax(s + cb)_k = exp(s_k)*exp(cb_k) /
sum_k' exp(s_k')*exp(cb_k'), so scaling V rows and the denominator by
exp(cb_k) is exactly the bias add.
"""

import numpy as np

import concourse.tile as tile
from concourse import bacc, bass_utils, mybir

F32 = mybir.dt.float32
F32R = mybir.dt.float32r
AF = mybir.ActivationFunctionType

B, S, H = 2, 2048, 1024
NH, HD = 16, 64
M = 256          # per-core projection dims (4 heads)
KC = H // 128    # 8 contraction chunks for the projections
ST = S // 128    # 16 key-position chunks
N_CORES = 8

_NC_CACHE = {}


def _attention_kernel(tc, reps=1, mode="full"):
    nc = tc.nc
    hsT = nc.dram_tensor("hsT", [H, S], F32R, kind="ExternalInput").ap()
    W3T = nc.dram_tensor("W3T", [H, 3 * M], F32R, kind="ExternalInput").ap()
    smalls = nc.dram_tensor("smalls", [128, 4 + ST], F32, kind="ExternalInput").ap()
    ctxu = nc.dram_tensor("ctxu", [4, HD + 1, S], F32, kind="ExternalOutput").ap()

    for _rep in range(reps):
      with (
        tc.tile_pool(name="const", bufs=1) as const,
        tc.tile_pool(name="big", bufs=1) as big,
      ):
        sm_sb = const.tile([128, 4 + ST], F32, tag="smalls", name="smalls")
        nc.scalar.dma_start(out=sm_sb[:], in_=smalls[:])
        bq_sb = sm_sb[:, 0:2]
        bk_sb = sm_sb[:, 2:4]
        expb_sb = sm_sb[:, 4:4 + ST]
        ones_sb = const.tile([128, 4], F32, tag="ones", name="ones")
        nc.vector.memset(ones_sb[:], 1.0)

        # One big DMA each for hs^T and the three weight matrices: a single
        # InstDMACopy fans out across all 16 SDMA engines (~400 GB/s) where
        # many small DMAs serialize on the HWDGE ring.
        hsT_big = big.tile([128, KC, S], F32R, tag="hsT", name="hsT_sb")
        nc.sync.dma_start(out=hsT_big[:], in_=hsT.rearrange("(c p) s -> p c s", p=128))
        w3_big = big.tile([128, KC, 3 * M], F32R, tag="w3", name="w3_sb")
        nc.scalar.dma_start(out=w3_big[:], in_=W3T.rearrange("(c p) m -> p c m", p=128))
        hsT_t = [hsT_big[:, k, :] for k in range(KC)]
        wq_t = [w3_big[:, k, 0:M] for k in range(KC)]
        wk_t = [w3_big[:, k, M:2 * M] for k in range(KC)]
        wv_t = [w3_big[:, k, 2 * M:3 * M] for k in range(KC)]

        # Persistent projection outputs.
        QT = [big.tile([128, S], F32R, tag=f"QT{t}", name=f"QT{t}") for t in range(2)]
        KT = [big.tile([128, S], F32R, tag=f"KT{t}", name=f"KT{t}") for t in range(2)]
        # V' with exp(bias) column interleaved: per key chunk, 4 head blocks
        # of [64 scaled V dims | expb] = 260 columns.
        Vp = [big.tile([128, 4, HD + 1], F32R, tag=f"Vp{s}", name=f"Vp{s}") for s in range(ST)]

        with (
            tc.tile_pool(name="pp", bufs=2, space="PSUM") as pp,
            tc.tile_pool(name="pt", bufs=2) as pt_pool,
            tc.tile_pool(name="cs", bufs=2) as cs_pool,
            tc.tile_pool(name="sc", bufs=1, space="PSUM") as sc_pool,
            tc.tile_pool(name="cx", bufs=1, space="PSUM") as cx_pool,
        ):

            def qk_chain(w_t, out_t, bias_sb, mt, sc):
                ps = pp.tile([128, 512], F32, tag="qk", name="qk")
                for k in range(KC):
                    nc.tensor.matmul(
                        ps[:],
                        w_t[k][:, mt * 128:(mt + 1) * 128],
                        hsT_t[k][:, sc * 512:(sc + 1) * 512],
                        start=(k == 0),
                        stop=(k == KC - 1),
                    )
                nc.vector.tensor_scalar_add(
                    out_t[mt][:, sc * 512:(sc + 1) * 512],
                    ps[:],
                    bias_sb[:, mt:mt + 1],
                )

            def v_chain(st):
                ps = pp.tile([128, M], F32, tag="qk", name="v")
                for k in range(KC):
                    nc.tensor.matmul(
                        ps[:],
                        hsT_t[k][:, st * 128:(st + 1) * 128],
                        wv_t[k][:],
                        start=(k == 0),
                        stop=(k == KC - 1),
                    )
                nc.vector.tensor_scalar_mul(
                    Vp[st][:, :, 0:HD],
                    ps[:].rearrange("p (h d) -> p h d", h=4),
                    expb_sb[:, st:st + 1],
                )
                nc.vector.tensor_scalar_mul(
                    Vp[st][:, :, HD:HD + 1],
                    ones_sb[:].rearrange("p (h d) -> p h d", h=4),
                    expb_sb[:, st:st + 1],
                )

            if mode == "dmaonly":
                dummy = const.tile([128, 1], F32, tag="dummy", name="dummy")
                nc.vector.tensor_copy(dummy[:], hsT_big[:, 0, 0:1].bitcast(F32))
                nc.vector.tensor_copy(dummy[:], w3_big[:, 0, 0:1].bitcast(F32))
                continue

            # Minimal prefix so head-pair 0 / sq-chunk 0 / kk=0 can start as
            # soon as possible ...
            qk_chain(wk_t, KT, bk_sb, 0, 0)
            qk_chain(wq_t, QT, bq_sb, 0, 0)
            v_chain(0)
            v_chain(1)
            # ... then the rest of the work pair-0 attention consumes early
            # (K columns and V chunks in kk order), and a background queue of
            # everything else, drained one chain per kk iteration so the PE
            # fills its slack under the ACT-bound attention loop without
            # starving it.
            qk_chain(wk_t, KT, bk_sb, 0, 1)
            v_chain(2)
            v_chain(3)
            # Remaining work is emitted *inside* the attention loops, always
            # in program order before its first consumer (Tile dependencies
            # follow program order - a consumer emitted before its producer
            # reads garbage).  Late projections fill the PE's slack under the
            # ACT-bound attention iterations.
            bg = [("k1", sc) for sc in range(4)] + [("q1", sc) for sc in range(4)]
            bg.reverse()  # pop() from the front

            def drain_bg(n):
                for _ in range(n):
                    if not bg:
                        return
                    kind, arg = bg.pop()
                    if kind == "v":
                        v_chain(arg)
                    elif kind == "k0":
                        qk_chain(wk_t, KT, bk_sb, 0, arg)
                    elif kind == "q0":
                        qk_chain(wq_t, QT, bq_sb, 0, arg)
                    elif kind == "k1":
                        qk_chain(wk_t, KT, bk_sb, 1, arg)
                    elif kind == "q1":
                        qk_chain(wq_t, QT, bq_sb, 1, arg)

            if mode == "projonly":
                drain_bg(len(bg))
                continue

            # Attention: pair p = local heads 2p, 2p+1 living on SBUF
            # partitions 0-63 / 64-127 of QT[p]/KT[p] - row-packed on PE.
            for p in range(2):
                if p == 1:
                    drain_bg(len(bg))
                for sqc in range(4):
                    if p == 0 and sqc >= 1:
                        qk_chain(wq_t, QT, bq_sb, 0, sqc)
                    sq = slice(sqc * 512, (sqc + 1) * 512)
                    cA = cx_pool.tile([HD + 1, 512], F32, tag="cA", name="cA")
                    cB = cx_pool.tile([HD + 1, 512], F32, tag="cB", name="cB")
                    for kk in range(8):
                        if p == 0 and sqc == 0 and kk >= 2:
                            v_chain(2 * kk)
                            v_chain(2 * kk + 1)
                            if kk in (4, 6):
                                qk_chain(wk_t, KT, bk_sb, 0, kk // 2)
                        sA = sc_pool.tile([128, 1024], F32, tag="sA", name="sA")
                        sB = sc_pool.tile([128, 1024], F32, tag="sB", name="sB")
                        for i in range(2):
                            kch = 2 * kk + i
                            ks = slice(kch * 128, (kch + 1) * 128)
                            nc.tensor.matmul(
                                sA[:, i * 512:(i + 1) * 512],
                                KT[p][0:64, ks],
                                QT[p][0:64, sq],
                            )
                            nc.tensor.matmul(
                                sB[:, i * 512:(i + 1) * 512],
                                KT[p][64:128, ks],
                                QT[p][64:128, sq],
                            )
                        if mode == "scoresonly":
                            dmy = pt_pool.tile([128, 1], F32, tag="dmy", name="dmy")
                            nc.vector.tensor_copy(dmy[:], sA[:, 0:1])
                            nc.vector.tensor_copy(dmy[:], sB[:, 0:1])
                            continue
                        pA = pt_pool.tile([128, 1024], F32R, tag="pA", name="pA")
                        pB = pt_pool.tile([128, 1024], F32R, tag="pB", name="pB")
                        nc.scalar.activation(pA[:], sA[:], AF.Exp, scale=0.125)
                        nc.scalar.activation(pB[:], sB[:], AF.Exp, scale=0.125)
                        if mode == "nopv":
                            dmy = pt_pool.tile([128, 1], F32, tag="dmy", name="dmy")
                            nc.vector.tensor_copy(dmy[:], pA[:, 0:1].bitcast(F32))
                            nc.vector.tensor_copy(dmy[:], pB[:, 0:1].bitcast(F32))
                            continue
                        for i in range(2):
                            kch = 2 * kk + i
                            flags = dict(
                                start=(kk == 0 and i == 0),
                                stop=(kk == 7 and i == 1),
                            )
                            nc.tensor.matmul(
                                cA[:],
                                Vp[kch][:, 2 * p, :],
                                pA[:, i * 512:(i + 1) * 512],
                                **flags,
                            )
                            nc.tensor.matmul(
                                cB[:],
                                Vp[kch][:, 2 * p + 1, :],
                                pB[:, i * 512:(i + 1) * 512],
                                **flags,
                            )
                        if kk % 2 == 1 and not (p == 0 and sqc == 0):
                            drain_bg(1)
                    if mode in ("scoresonly", "nopv"):
                        continue
                    oA = cs_pool.tile([HD + 1, 512], F32, tag="oA", name="oA")
                    oB = cs_pool.tile([HD + 1, 512], F32, tag="oB", name="oB")
                    nc.vector.tensor_copy(oA[:], cA[:])
                    nc.vector.tensor_copy(oB[:], cB[:])
                    nc.sync.dma_start(out=ctxu[2 * p, :, sq], in_=oA[:])
                    nc.sync.dma_start(out=ctxu[2 * p + 1, :, sq], in_=oB[:])
            drain_bg(len(bg))


def build_nc(reps=1, mode="full"):
    key = (reps, mode)
    if key in _NC_CACHE:
        return _NC_CACHE[key]
    nc = bacc.Bacc("TRN2", target_bir_lowering=False, debug=False)
    with tile.TileContext(nc) as tc:
        _attention_kernel(tc, reps=reps, mode=mode)
    nc.compile()
    _NC_CACHE[key] = nc
    return nc


def make_in_maps(hidden_states, causal_bias, Wq, bq, Wk, bk, Wv, bv):
    hs = np.ascontiguousarray(np.asarray(hidden_states, dtype=np.float32))
    cb = np.asarray(causal_bias, dtype=np.float32)
    expb = np.exp(cb).reshape(ST, 128).T.copy()  # [128, ST]
    hsT = [np.ascontiguousarray(hs[b].T) for b in range(B)]
    in_maps = []
    for c in range(N_CORES):
        b, g = divmod(c, 4)
        sl = slice(g * M, (g + 1) * M)
        w3 = np.concatenate([
            np.asarray(Wq, np.float32)[sl].T,
            np.asarray(Wk, np.float32)[sl].T,
            np.asarray(Wv, np.float32)[sl].T,
        ], axis=1)
        sm = np.concatenate([
            np.asarray(bq, np.float32)[sl].reshape(2, 128).T,
            np.asarray(bk, np.float32)[sl].reshape(2, 128).T,
            expb,
        ], axis=1)
        in_maps.append({
            "hsT": hsT[b],
            "W3T": np.ascontiguousarray(w3),
            "smalls": np.ascontiguousarray(sm),
        })
    return in_maps


def gather_output(results, bv):
    bv = np.asarray(bv, np.float32)
    out = np.empty((B, S, H), np.float32)
    for c in range(N_CORES):
        b, g = divmod(c, 4)
        sl = slice(g * M, (g + 1) * M)
        ctxu = results[c]["ctxu"]  # [4, 65, S]
        ctx = (ctxu[:, :HD, :] / ctxu[:, HD:HD + 1, :]).transpose(2, 0, 1)
        out[b, :, sl] = ctx.reshape(S, M) + bv[sl][None, :]
    return out


def kernel(hidden_states, causal_bias, Wq, bq, Wk, bk, Wv, bv):
    nc = build_nc()
    in_maps = make_in_maps(hidden_states, causal_bias, Wq, bq, Wk, bk, Wv, bv)
    res = bass_utils.run_bass_kernel_spmd(nc, in_maps, core_ids=list(range(N_CORES)))
    return gather_output(res.results, bv)


# revision 31
# speedup vs baseline: 1.0188x; 1.0188x over previous
"""BertSelfAttention on 8 Trainium2 NeuronCores (Bass/Tile, SPMD, no collectives).

Problem: hidden_states [2, 2048, 1024], 16 heads x 64 dims, causal_bias added
along the key axis before softmax.

Sharding: core c handles batch b = c//4 and head-group g = c%4 (4 heads, i.e.
256 of the 1024 projection dims).  Pure SPMD - every core runs the same
program on its own slice; the host does the (free) slicing / transposes and
the final gather.

Per-core device algorithm (all matmuls in fp32r = full-rate fp32):
  QT[m, s] = Wq_g @ hsT + bq   (m = 256 local head dims, s = 2048 positions)
  KT[m, s] = Wk_g @ hsT + bk
  V [s, m] = (hs @ Wv_g.T) * expb[s]   (expb = exp(causal_bias), no bv)
  per head h (2 row-packed pairs):
    sT[k, sq]  = KT_h.T @ QT_h          (scores transposed, k = key pos)
    P [k, sq]  = exp(sT * 0.125)        (bias folded in via expb; no max
                                         subtraction needed: |s/8| < ~3)
    ctxu[65, sq] += [V'_h | expb].T @ P (rows 0..63 = unnormalized ctx^T,
                                         row 64 = softmax denominator)
  DMA ctxu to DRAM.
Host: ctx = (ctxu[:64] / ctxu[64]).T + bv  and scatter into [B, S, H].

The exp(bias) folding works because softmax(s + cb)_k = exp(s_k)*exp(cb_k) /
sum_k' exp(s_k')*exp(cb_k'), so scaling V rows and the denominator by
exp(cb_k) is exactly the bias add.
"""

import numpy as np

import concourse.tile as tile
from concourse import bacc, bass_utils, mybir

F32 = mybir.dt.float32
F32R = mybir.dt.float32r
AF = mybir.ActivationFunctionType

B, S, H = 2, 2048, 1024
NH, HD = 16, 64
M = 256          # per-core projection dims (4 heads)
KC = H // 128    # 8 contraction chunks for the projections
ST = S // 128    # 16 key-position chunks
N_CORES = 8

_NC_CACHE = {}


def _attention_kernel(tc, reps=1, mode="full"):
    nc = tc.nc
    hsT = nc.dram_tensor("hsT", [H, S], F32R, kind="ExternalInput").ap()
    W3T = nc.dram_tensor("W3T", [H, 3 * M], F32R, kind="ExternalInput").ap()
    smalls = nc.dram_tensor("smalls", [128, 4 + ST], F32, kind="ExternalInput").ap()
    ctxu = nc.dram_tensor("ctxu", [4, HD + 1, S], F32, kind="ExternalOutput").ap()

    for _rep in range(reps):
      with (
        tc.tile_pool(name="const", bufs=1) as const,
        tc.tile_pool(name="big", bufs=1) as big,
      ):
        sm_sb = const.tile([128, 4 + ST], F32, tag="smalls", name="smalls")
        bq_sb = sm_sb[:, 0:2]
        bk_sb = sm_sb[:, 2:4]
        expb_sb = sm_sb[:, 4:4 + ST]
        ones_sb = const.tile([128, 4], F32, tag="ones", name="ones")
        nc.vector.memset(ones_sb[:], 1.0)

        # Batched input DMAs (a single InstDMACopy fans across all 16 SDMA
        # engines at ~400 GB/s; many small DMAs serialize on one DGE ring),
        # split in balanced halves across the two HWDGE rings (SP + ACT) so
        # the ~11 MB of inputs land in ~15 us instead of queueing ~25 us on
        # one ring.  Weights go first: every projection chain needs them.
        half = KC // 2
        hsT_big = big.tile([128, KC, S], F32R, tag="hsT", name="hsT_sb")
        hsT_r = hsT.rearrange("(c p) s -> p c s", p=128)
        w3_big = big.tile([128, KC, 3 * M], F32R, tag="w3", name="w3_sb")
        w3_r = W3T.rearrange("(c p) m -> p c m", p=128)
        nc.sync.dma_start(out=w3_big[:, 0:half, :], in_=w3_r[:, 0:half, :])
        nc.scalar.dma_start(out=w3_big[:, half:KC, :], in_=w3_r[:, half:KC, :])
        nc.sync.dma_start(out=hsT_big[:, 0:half, :], in_=hsT_r[:, 0:half, :])
        nc.scalar.dma_start(out=hsT_big[:, half:KC, :], in_=hsT_r[:, half:KC, :])
        # the tiny strided bias/expb transfer queues last so it never delays
        # the bulk transfers on either HWDGE ring (its first consumer, the
        # first chain's DVE copy, runs well after the bulk lands)
        nc.sync.dma_start(out=sm_sb[:], in_=smalls[:])
        hsT_t = [hsT_big[:, k, :] for k in range(KC)]
        wq_t = [w3_big[:, k, 0:M] for k in range(KC)]
        wk_t = [w3_big[:, k, M:2 * M] for k in range(KC)]
        wv_t = [w3_big[:, k, 2 * M:3 * M] for k in range(KC)]

        # Persistent projection outputs.
        QT = [big.tile([128, S], F32R, tag=f"QT{t}", name=f"QT{t}") for t in range(2)]
        KT = [big.tile([128, S], F32R, tag=f"KT{t}", name=f"KT{t}") for t in range(2)]
        # V' with exp(bias) column interleaved: per key chunk, 4 head blocks
        # of [64 scaled V dims | expb] = 260 columns.
        Vp = [big.tile([128, 4, HD + 1], F32R, tag=f"Vp{s}", name=f"Vp{s}") for s in range(ST)]

        with (
            tc.tile_pool(name="pp", bufs=2, space="PSUM") as pp,
            tc.tile_pool(name="pt", bufs=2) as pt_pool,
            tc.tile_pool(name="cs", bufs=2) as cs_pool,
            tc.tile_pool(name="sc", bufs=1, space="PSUM") as sc_pool,
            tc.tile_pool(name="cx", bufs=1, space="PSUM") as cx_pool,
        ):

            def qk_chain(w_t, out_t, bias_sb, mt, sc):
                ps = pp.tile([128, 512], F32, tag="qk", name="qk")
                for k in range(KC):
                    nc.tensor.matmul(
                        ps[:],
                        w_t[k][:, mt * 128:(mt + 1) * 128],
                        hsT_t[k][:, sc * 512:(sc + 1) * 512],
                        start=(k == 0),
                        stop=(k == KC - 1),
                    )
                nc.vector.tensor_scalar_add(
                    out_t[mt][:, sc * 512:(sc + 1) * 512],
                    ps[:],
                    bias_sb[:, mt:mt + 1],
                )

            def v_chain(st):
                ps = pp.tile([128, M], F32, tag="qk", name="v")
                for k in range(KC):
                    nc.tensor.matmul(
                        ps[:],
                        hsT_t[k][:, st * 128:(st + 1) * 128],
                        wv_t[k][:],
                        start=(k == 0),
                        stop=(k == KC - 1),
                    )
                nc.vector.tensor_scalar_mul(
                    Vp[st][:, :, 0:HD],
                    ps[:].rearrange("p (h d) -> p h d", h=4),
                    expb_sb[:, st:st + 1],
                )
                nc.vector.tensor_scalar_mul(
                    Vp[st][:, :, HD:HD + 1],
                    ones_sb[:].rearrange("p (h d) -> p h d", h=4),
                    expb_sb[:, st:st + 1],
                )

            if mode == "dmaonly":
                dummy = const.tile([128, 1], F32, tag="dummy", name="dummy")
                nc.vector.tensor_copy(dummy[:], hsT_big[:, 0, 0:1].bitcast(F32))
                nc.vector.tensor_copy(dummy[:], w3_big[:, 0, 0:1].bitcast(F32))
                continue

            # Minimal prefix so head-pair 0 / sq-chunk 0 / kk=0 can start as
            # soon as possible ...
            qk_chain(wk_t, KT, bk_sb, 0, 0)
            qk_chain(wq_t, QT, bq_sb, 0, 0)
            v_chain(0)
            v_chain(1)
            # ... then the rest of the work pair-0 attention consumes early
            # (K columns and V chunks in kk order), and a background queue of
            # everything else, drained one chain per kk iteration so the PE
            # fills its slack under the ACT-bound attention loop without
            # starving it.
            qk_chain(wk_t, KT, bk_sb, 0, 1)
            v_chain(2)
            v_chain(3)
            # Remaining work is emitted *inside* the attention loops, always
            # in program order before its first consumer (Tile dependencies
            # follow program order - a consumer emitted before its producer
            # reads garbage).  Late projections fill the PE's slack under the
            # ACT-bound attention iterations.
            bg = [("k1", sc) for sc in range(4)] + [("q1", sc) for sc in range(4)]
            bg.reverse()  # pop() from the front

            def drain_bg(n):
                for _ in range(n):
                    if not bg:
                        return
                    kind, arg = bg.pop()
                    if kind == "v":
                        v_chain(arg)
                    elif kind == "k0":
                        qk_chain(wk_t, KT, bk_sb, 0, arg)
                    elif kind == "q0":
                        qk_chain(wq_t, QT, bq_sb, 0, arg)
                    elif kind == "k1":
                        qk_chain(wk_t, KT, bk_sb, 1, arg)
                    elif kind == "q1":
                        qk_chain(wq_t, QT, bq_sb, 1, arg)

            if mode == "projonly":
                drain_bg(len(bg))
                continue

            # Attention: pair p = local heads 2p, 2p+1 living on SBUF
            # partitions 0-63 / 64-127 of QT[p]/KT[p] - row-packed on PE.
            for p in range(2):
                if p == 1:
                    drain_bg(len(bg))
                for sqc in range(4):
                    if p == 0 and sqc >= 1:
                        qk_chain(wq_t, QT, bq_sb, 0, sqc)
                    sq = slice(sqc * 512, (sqc + 1) * 512)
                    cA = cx_pool.tile([HD + 1, 512], F32, tag="cA", name="cA")
                    cB = cx_pool.tile([HD + 1, 512], F32, tag="cB", name="cB")
                    for kk in range(8):
                        if p == 0 and sqc == 0 and kk >= 2:
                            v_chain(2 * kk)
                            v_chain(2 * kk + 1)
                            if kk in (4, 6):
                                qk_chain(wk_t, KT, bk_sb, 0, kk // 2)
                        sA = sc_pool.tile([128, 1024], F32, tag="sA", name="sA")
                        sB = sc_pool.tile([128, 1024], F32, tag="sB", name="sB")
                        for i in range(2):
                            kch = 2 * kk + i
                            ks = slice(kch * 128, (kch + 1) * 128)
                            nc.tensor.matmul(
                                sA[:, i * 512:(i + 1) * 512],
                                KT[p][0:64, ks],
                                QT[p][0:64, sq],
                            )
                            nc.tensor.matmul(
                                sB[:, i * 512:(i + 1) * 512],
                                KT[p][64:128, ks],
                                QT[p][64:128, sq],
                            )
                        if mode == "scoresonly":
                            dmy = pt_pool.tile([128, 1], F32, tag="dmy", name="dmy")
                            nc.vector.tensor_copy(dmy[:], sA[:, 0:1])
                            nc.vector.tensor_copy(dmy[:], sB[:, 0:1])
                            continue
                        pA = pt_pool.tile([128, 1024], F32R, tag="pA", name="pA")
                        pB = pt_pool.tile([128, 1024], F32R, tag="pB", name="pB")
                        nc.scalar.activation(pA[:], sA[:], AF.Exp, scale=0.125)
                        nc.scalar.activation(pB[:], sB[:], AF.Exp, scale=0.125)
                        if mode == "nopv":
                            dmy = pt_pool.tile([128, 1], F32, tag="dmy", name="dmy")
                            nc.vector.tensor_copy(dmy[:], pA[:, 0:1].bitcast(F32))
                            nc.vector.tensor_copy(dmy[:], pB[:, 0:1].bitcast(F32))
                            continue
                        for i in range(2):
                            kch = 2 * kk + i
                            flags = dict(
                                start=(kk == 0 and i == 0),
                                stop=(kk == 7 and i == 1),
                            )
                            nc.tensor.matmul(
                                cA[:],
                                Vp[kch][:, 2 * p, :],
                                pA[:, i * 512:(i + 1) * 512],
                                **flags,
                            )
                            nc.tensor.matmul(
                                cB[:],
                                Vp[kch][:, 2 * p + 1, :],
                                pB[:, i * 512:(i + 1) * 512],
                                **flags,
                            )
                        if kk % 2 == 1 and not (p == 0 and sqc == 0):
                            drain_bg(1)
                    if mode in ("scoresonly", "nopv"):
                        continue
                    oA = cs_pool.tile([HD + 1, 512], F32, tag="oA", name="oA")
                    oB = cs_pool.tile([HD + 1, 512], F32, tag="oB", name="oB")
                    nc.vector.tensor_copy(oA[:], cA[:])
                    nc.vector.tensor_copy(oB[:], cB[:])
                    nc.sync.dma_start(out=ctxu[2 * p, :, sq], in_=oA[:])
                    nc.sync.dma_start(out=ctxu[2 * p + 1, :, sq], in_=oB[:])
            drain_bg(len(bg))


def build_nc(reps=1, mode="full"):
    key = (reps, mode)
    if key in _NC_CACHE:
        return _NC_CACHE[key]
    nc = bacc.Bacc("TRN2", target_bir_lowering=False, debug=False)
    with tile.TileContext(nc) as tc:
        _attention_kernel(tc, reps=reps, mode=mode)
    nc.compile()
    _NC_CACHE[key] = nc
    return nc


def make_in_maps(hidden_states, causal_bias, Wq, bq, Wk, bk, Wv, bv):
    hs = np.ascontiguousarray(np.asarray(hidden_states, dtype=np.float32))
    cb = np.asarray(causal_bias, dtype=np.float32)
    expb = np.exp(cb).reshape(ST, 128).T.copy()  # [128, ST]
    hsT = [np.ascontiguousarray(hs[b].T) for b in range(B)]
    in_maps = []
    for c in range(N_CORES):
        b, g = divmod(c, 4)
        sl = slice(g * M, (g + 1) * M)
        w3 = np.concatenate([
            np.asarray(Wq, np.float32)[sl].T,
            np.asarray(Wk, np.float32)[sl].T,
            np.asarray(Wv, np.float32)[sl].T,
        ], axis=1)
        sm = np.concatenate([
            np.asarray(bq, np.float32)[sl].reshape(2, 128).T,
            np.asarray(bk, np.float32)[sl].reshape(2, 128).T,
            expb,
        ], axis=1)
        in_maps.append({
            "hsT": hsT[b],
            "W3T": np.ascontiguousarray(w3),
            "smalls": np.ascontiguousarray(sm),
        })
    return in_maps


def gather_output(results, bv):
    bv = np.asarray(bv, np.float32)
    out = np.empty((B, S, H), np.float32)
    for c in range(N_CORES):
        b, g = divmod(c, 4)
        sl = slice(g * M, (g + 1) * M)
        ctxu = results[c]["ctxu"]  # [4, 65, S]
        ctx = (ctxu[:, :HD, :] / ctxu[:, HD:HD + 1, :]).transpose(2, 0, 1)
        out[b, :, sl] = ctx.reshape(S, M) + bv[sl][None, :]
    return out


def kernel(hidden_states, causal_bias, Wq, bq, Wk, bk, Wv, bv):
    nc = build_nc()
    in_maps = make_in_maps(hidden_states, causal_bias, Wq, bq, Wk, bk, Wv, bv)
    res = bass_utils.run_bass_kernel_spmd(nc, in_maps, core_ids=list(range(N_CORES)))
    return gather_output(res.results, bv)
